# revision 25
# baseline (speedup 1.0000x reference)
"""Trainium2 Bass kernel for nn_CircuitChannel (20-qubit statevector circuit).

Strategy: batch-parallel — BATCH=8 == n_cores, one full 2^20 statevector per
NeuronCore. Key algebraic reduction vs the complex-gate formulation:
RX(theta) = S^dag RY(theta) S with S = diag(1, i) per qubit, and both
S_global = (x)diag(1,i)^{tensor 20} and the CZ-ring sign are diagonal, so all
S factors telescope through the circuit:

    circuit = S^dag . Prod_l [ D_CZ . (x)RY_l ] . S

S / S^dag are elementwise i^popcount multiplies folded into HOST pre/post
processing (numpy), so every device gate pass becomes a REAL orthogonal
128x128 matrix (7-qubit RY tensor-product group) — HALF the PE streaming
work of the complex formulation (one PE column per real value).

The terminal qubit-0 measurement is also computed on host from the returned
final state (identical fp16 values, so numerically equivalent), leaving the
device program as: load -> 12 real gate passes -> store.

Stage structure per layer: two transposing-matmul passes (TM6/TM0:
stationary = state block, moving = gate; result lands transposed in PSUM,
swapping a 7-bit free-axis group onto the partition axis) + one plain pass
(PM: stationary = gate, moving = state columns). State and gates are fp16
(full-rate PE; ~1e-4 quantization per pass); PSUM accumulates fp32.

HW-measured design choices (the instruction cost model misses these):
 - GpSimd cannot access PSUM (BIR verifier), so PSUM evacuation runs on
   DVE+ACT only, with per-stage engine patterns.
 - Strided evacuation writes are ~2x slower than contiguous on HW, so the
   TM0 pass writes its transpose CONTIGUOUSLY, making it a 3-cycle bit
   permutation instead of a closing swap; the final non-identity bit
   layout is un-permuted on the host (free).
 - Direct tensor_tensor sign-multiplies out of PSUM are ~3x a plain copy
   on HW, so PM evacuates with plain DVE/ACT copies and the CZ sign lands
   as deferred SBUF->SBUF multiplies on the otherwise-idle GpSimd engine,
   overlapped with the next stage.
 - Dual alternating PSUM pools decouple consecutive stages' buffer FIFOs.
"""
import sys
sys.path.insert(0, "/opt/trn_rl_repo")
import numpy as np

N = 20
DIM = 1 << N
BATCH = 8
NLAYERS = 4

STAGES = [
    ("TM6", 0), ("TM0", 0), ("PM", 0),
    ("TM6", 1), ("TM0", 1), ("PM", 1),
    ("TM6", 2), ("TM0", 2), ("PM", 2),
    ("TM6", 3), ("TM0", 3), ("PM", 3),
]

# Evacuation engine assignment. TM stages: 32 tiles of [128,512];
# D = DVE copy, A = ACT copy, P = GpSimd copy.
# PM stages: 32 (chunk, plane) ops; V = DVE sign-multiply, G = GpSimd mult.

def _spread(counts, n):
    """Evenly interleave engine tokens with the given counts over n slots."""
    acc = {k: 0.0 for k in counts}
    out = []
    for _ in range(n):
        for k in counts:
            acc[k] += counts[k] / n
        k = max(acc, key=lambda x: acc[x])
        acc[k] -= 1.0
        out.append(k)
    return "".join(out)


# GpSimd cannot access PSUM (BIR verifier), so evacuation is DVE/ACT only.
# PSUM is fp32-only on TRN2, so evacuation converts f32->f16 on DVE/ACT.
# TM: 16 tiles/stage, tokens D (DVE copy) / A (ACT copy).
# PM: 16 (chunk, plane) ops; V = DVE sign-mult, c = ACT copy + GpSimd
# deferred SBUF mult, d = ACT copy + DVE deferred SBUF mult.
TM_PAT = _spread({"D": 12, "A": 4}, 16)
PM_PAT = _spread({"C": 12, "c": 4}, 16)
SINGLE_POOL = True
PSUM_W = 1024
STAGES_OVERRIDE = None  # timing diagnostics: e.g. [("TM6",0)]*3 per rep
TM0_CONTIG = False      # diagnostic: TM0 with contiguous (TM6-style) evac
PM_NOSIGN = False       # diagnostic: PM with plain copies (no sign mult)
# Timing-diagnostic mode: replace full-width evacuations with tiny sampled
# copies (keeps every matmul live + the dependency structure, breaks data).
SAMPLED_EVAC = False


# ------------------------- host-side plan -------------------------

def _ry(theta):
    c, s = np.cos(theta / 2), np.sin(theta / 2)
    return np.array([[c, -s], [s, c]], dtype=np.float64)


def _cz_sign_canonical():
    idx = np.arange(DIM, dtype=np.int64)
    bits = (idx[None, :] >> (N - 1 - np.arange(N)[:, None])) & 1
    par = np.sum(bits[:-1] * bits[1:], axis=0) % 2
    return (1 - 2 * par).astype(np.float64)


def _apply_sigma(layout, t):
    l = list(layout)
    if t == 6:
        return l[13:20] + l[7:13] + l[0:7]
    # TM0 with contiguous evacuation: part' = old free-top-7, free' =
    # [old free-bottom-6 | gated old part] (3-cycle, does not close; the
    # host un-permutes the final state).
    return l[7:14] + l[14:20] + l[0:7]


def final_layout():
    layout = list(range(N))
    for stype, _ in STAGES:
        if stype == "TM6":
            layout = _apply_sigma(layout, 6)
        elif stype == "TM0":
            layout = _apply_sigma(layout, 0)
    return layout


def _sign_in_layout(s_canon, layout):
    pf = np.arange(DIM, dtype=np.int64)
    idx = np.zeros(DIM, dtype=np.int64)
    for j in range(N):
        bit = (pf >> (N - 1 - j)) & 1
        idx |= bit << (N - 1 - layout[j])
    return s_canon[idx].reshape(128, 8192).astype(np.float16)


def build_plan(thetas):
    s_canon = _cz_sign_canonical()
    layout = list(range(N))
    plan = []
    done = set()
    cur_layer = -1
    for stype, layer in STAGES:
        if layer != cur_layer:
            assert cur_layer == -1 or len(done) == N, (cur_layer, len(done))
            done = set()
            cur_layer = layer
        U = np.array([[1.0]])
        for j in range(7):
            q = layout[j]
            g = np.eye(2) if q in done else _ry(thetas[layer, q])
            done.add(q)
            U = np.kron(U, g)
        st = dict(type=stype, U=U)
        if stype == "TM6":
            layout = _apply_sigma(layout, 6)
        elif stype == "TM0":
            layout = _apply_sigma(layout, 0)
        else:
            st["sign"] = _sign_in_layout(s_canon, layout)
        plan.append(st)
    assert len(done) == N
    return plan


def stage_weights(plan):
    """Per-stage [128,128] fp16 weight = G.T (real gate, both TM and PM)."""
    return [np.ascontiguousarray(st["U"].T.astype(np.float16)) for st in plan]


_PC4 = None


def popcount_mod4():
    global _PC4
    if _PC4 is None:
        idx = np.arange(DIM, dtype=np.int64)
        pc = np.zeros(DIM, dtype=np.int64)
        for j in range(N):
            pc += (idx >> j) & 1
        _PC4 = (pc % 4).astype(np.int8)
    return _PC4


# ------------------------- device program -------------------------

_NC_CACHE = {}


def _build_nc(reps=1):
    import concourse.bacc as bacc
    import concourse.mybir as mybir
    import concourse.tile as tile

    F32 = mybir.dt.float32
    F16 = mybir.dt.float16
    AX = mybir.AluOpType

    nc = bacc.Bacc(None)
    pr = nc.declare_dram_parameter("pr", [128, 8192], F16, isOutput=False)
    pi = nc.declare_dram_parameter("pi", [128, 8192], F16, isOutput=False)
    wps = [nc.declare_dram_parameter(f"w{s}", [128, 128], F16, isOutput=False)
           for s in range(len(STAGES))]
    sgs = [nc.declare_dram_parameter(f"sg{l}", [128, 8192], F16, isOutput=False)
           for l in range(NLAYERS)]
    out = nc.declare_dram_parameter("out", [128, 16384], F16, isOutput=True)

    with tile.TileContext(nc) as tc:
        with (
            tc.tile_pool(name="st", bufs=1) as stp,
            tc.tile_pool(name="wp", bufs=1) as wp,
            tc.tile_pool(name="sgp", bufs=1) as sgp,
            tc.tile_pool(name="pstmA", bufs=4, space="PSUM") as pstmA,
            tc.tile_pool(name="pstmB", bufs=4, space="PSUM") as pstmB,
        ):
            Af = stp.tile([128, 16384], F16, tag="A")
            Bf = stp.tile([128, 16384], F16, tag="B")
            A = Af[:].rearrange("p (c f) -> p c f", c=2)
            Bv = Bf[:].rearrange("p (c f) -> p c f", c=2)
            sgt = [sgp.tile([128, 8192], F16, tag=f"sg{l}", name=f"sg{l}")
                   for l in range(NLAYERS)]
            wts = [wp.tile([128, 128], F16, tag=f"w{s}", name=f"wt{s}")
                   for s in range(len(STAGES))]

            # Spread input DMAs across four trigger queues: a single queue's
            # bandwidth would pace stage 0 (state) and the first PM (sign
            # tables) well below compute speed.
            QS = [nc.sync, nc.gpsimd, nc.scalar]
            for s in range(len(STAGES)):
                nc.gpsimd.dma_start(wts[s][:], wps[s][:])
            # load state (chunked so stage 0 can start early)
            for ch in range(8):
                sl = slice(ch * 1024, (ch + 1) * 1024)
                QS[(2 * ch) % 3].dma_start(A[:, 0, sl], pr[:, sl])
                QS[(2 * ch + 1) % 3].dma_start(A[:, 1, sl], pi[:, sl])
            k = 0
            for l in range(NLAYERS):
                for ch in range(4):
                    sl = slice(ch * 2048, (ch + 1) * 2048)
                    QS[k % 3].dma_start(sgt[l][:, sl], sgs[l][:, sl])
                    k += 1

            ENG = {"D": nc.vector, "A": nc.scalar, "V": nc.vector}

            def tm6_stage(src, dst, w, pstm):
                for t in range(16):
                    p = pstm.tile([128, PSUM_W], F32, tag="mm", name="pt")
                    for b in range(4):
                        blk = t * 4 + b
                        xr = src[:, 0, blk * 128:(blk + 1) * 128]
                        xi = src[:, 1, blk * 128:(blk + 1) * 128]
                        nc.tensor.matmul(p[:, b * 256:b * 256 + 128], xr, w[:],
                                         start=True, stop=True)
                        nc.tensor.matmul(p[:, b * 256 + 128:b * 256 + 256], xi,
                                         w[:], start=True, stop=True)
                    pv = p[:].rearrange("p (b c x) -> p b c x", b=4, c=2)
                    dv = dst[:, :, t * 512:(t + 1) * 512].rearrange(
                        "p c (b x) -> p b c x", b=4)
                    if SAMPLED_EVAC:
                        nc.vector.tensor_copy(dv[:, :, :, 0:2], pv[:, :, :, 0:2])
                        continue
                    e = ENG[TM_PAT[t]]
                    if e is nc.scalar:
                        e.copy(dv, pv)
                    else:
                        e.tensor_copy(dv, pv)

            def tm0_stage(src, dst, w, pstm):
                srcr = src[:, 0, :].rearrange("p (w l) -> p l w", l=64)
                srci = src[:, 1, :].rearrange("p (w l) -> p l w", l=64)
                for t in range(16):
                    p = pstm.tile([128, PSUM_W], F32, tag="mm", name="pt")
                    for b in range(4):
                        blk = t * 4 + b
                        nc.tensor.matmul(p[:, b * 256:b * 256 + 128],
                                         srcr[:, blk, :], w[:],
                                         start=True, stop=True)
                        nc.tensor.matmul(p[:, b * 256 + 128:b * 256 + 256],
                                         srci[:, blk, :], w[:],
                                         start=True, stop=True)
                    pv = p[:].rearrange("p (b c x) -> p b c x", b=4, c=2)
                    dv = dst[:, :, t * 512:(t + 1) * 512].rearrange(
                        "p c (b x) -> p b c x", b=4)
                    if SAMPLED_EVAC:
                        nc.vector.tensor_copy(dv[:, :, :, 0:2], pv[:, :, :, 0:2])
                        continue
                    e = ENG[TM_PAT[t]]
                    if e is nc.scalar:
                        e.copy(dv, pv)
                    else:
                        e.tensor_copy(dv, pv)

            def pm_stage(src, dst, w, sg, pstm):
                deferred = []
                for ch in range(8):
                    sl = slice(ch * 1024, (ch + 1) * 1024)
                    pre = pstm.tile([128, 1024], F32, tag="mm", name="pt")
                    pim = pstm.tile([128, 1024], F32, tag="mm", name="pt")
                    for h in (0, 1):
                        msl = slice(ch * 1024 + h * 512,
                                    ch * 1024 + h * 512 + 512)
                        dsl = slice(h * 512, h * 512 + 512)
                        nc.tensor.matmul(pre[:, dsl], w[:], src[:, 0, msl],
                                         start=True, stop=True)
                        nc.tensor.matmul(pim[:, dsl], w[:], src[:, 1, msl],
                                         start=True, stop=True)
                    for plane, ptile in ((0, pre), (1, pim)):
                        if SAMPLED_EVAC:
                            nc.vector.tensor_copy(dst[:, plane, sl][:, 0:16],
                                                  ptile[:, 0:16])
                            continue
                        code = PM_PAT[ch * 2 + plane]
                        if code == "C":
                            nc.vector.tensor_copy(dst[:, plane, sl], ptile[:])
                        else:
                            nc.scalar.copy(dst[:, plane, sl], ptile[:])
                        if not PM_NOSIGN:
                            deferred.append((code, plane, sl))
                for k, (code, plane, sl) in enumerate(deferred):
                    nc.gpsimd.tensor_tensor(dst[:, plane, sl],
                                            dst[:, plane, sl],
                                            sg[:, sl], op=AX.mult)

            cur, nxt = A, Bv
            stage_list = STAGES_OVERRIDE if STAGES_OVERRIDE is not None else STAGES
            for _rep in range(reps):
                for s, (stype, layer) in enumerate(stage_list):
                    pool = pstmA if (SINGLE_POOL or s % 2 == 0) else pstmB
                    if stype == "PM":
                        pm_stage(cur, nxt, wts[s], sgt[layer], pool)
                    elif stype == "TM6":
                        tm6_stage(cur, nxt, wts[s], pool)
                    else:
                        tm0_stage(cur, nxt, wts[s], pool)
                    cur, nxt = nxt, cur
            if cur is not A:
                cur, nxt = nxt, cur  # diagnostics only: force A for the store

            for k in range(16):
                sl = slice(k * 1024, (k + 1) * 1024)
                nc.sync.dma_start(out[:, sl], Af[:, sl])
    nc.compile()
    return nc


def _get_nc(reps=1):
    if reps not in _NC_CACHE:
        _NC_CACHE[reps] = _build_nc(reps)
    return _NC_CACHE[reps]


# ------------------------- entry point -------------------------

def make_in_maps(psi_re, psi_im, thetas, u):
    """Host pre-processing: S-basis transform (i^popcount), fp16 cast,
    plan/weight/sign construction. Returns per-core input maps."""
    psi_re = np.asarray(psi_re, dtype=np.float32)
    psi_im = np.asarray(psi_im, dtype=np.float32)
    thetas = np.asarray(thetas, dtype=np.float32)

    plan = build_plan(thetas.astype(np.float64))
    ws = stage_weights(plan)
    signs = [st["sign"] for st in plan if st["type"] == "PM"]
    k = popcount_mod4()

    re_eff = np.where(k == 0, psi_re,
                      np.where(k == 1, -psi_im,
                               np.where(k == 2, -psi_re, psi_im)))
    im_eff = np.where(k == 0, psi_im,
                      np.where(k == 1, psi_re,
                               np.where(k == 2, -psi_im, -psi_re)))
    re16 = re_eff.astype(np.float16).reshape(BATCH, 128, 8192)
    im16 = im_eff.astype(np.float16).reshape(BATCH, 128, 8192)

    in_maps = []
    for b in range(BATCH):
        m = {"pr": re16[b], "pi": im16[b]}
        for s in range(len(STAGES)):
            m[f"w{s}"] = ws[s]
        for l in range(NLAYERS):
            m[f"sg{l}"] = signs[l]
        in_maps.append(m)
    return in_maps


_PERM = None


def _final_perm():
    """dev-flat-index -> canonical-index map for the final bit layout."""
    global _PERM
    if _PERM is None:
        lay = final_layout()
        pf = np.arange(DIM, dtype=np.int64)
        idx = np.zeros(DIM, dtype=np.int64)
        for j in range(N):
            bit = (pf >> (N - 1 - j)) & 1
            idx |= bit << (N - 1 - lay[j])
        _PERM = idx
    return _PERM


def postprocess(dev_outs, u):
    """Host post-processing: un-permute the device bit layout, then
    qubit-0 measurement + projection/normalization from the S-basis state,
    then S^dag back-transform."""
    u = np.asarray(u, dtype=np.float64)
    k = popcount_mod4()
    perm = _final_perm()
    res = np.empty((BATCH, DIM, 2), dtype=np.float32)
    half = DIM // 2
    for b in range(BATCH):
        o = dev_outs[b]  # [128, 16384] fp16
        fr = np.empty(DIM, dtype=np.float64)
        fi = np.empty(DIM, dtype=np.float64)
        fr[perm] = o[:, :8192].astype(np.float64).reshape(DIM)
        fi[perm] = o[:, 8192:].astype(np.float64).reshape(DIM)
        nrm2 = np.sum(fr * fr + fi * fi)
        p0 = np.sum(fr[:half] ** 2 + fi[:half] ** 2) / nrm2
        m = 0 if u[b] < p0 else 1
        p = p0 if m == 0 else 1.0 - p0
        s = 1.0 / np.sqrt(p * nrm2)
        if m == 0:
            fr[half:] = 0.0
            fi[half:] = 0.0
        else:
            fr[:half] = 0.0
            fi[:half] = 0.0
        fr *= s
        fi *= s
        # S^dag: multiply by (-i)^k
        re_o = np.where(k == 0, fr, np.where(k == 1, fi,
                        np.where(k == 2, -fr, -fi)))
        im_o = np.where(k == 0, fi, np.where(k == 1, -fr,
                        np.where(k == 2, -fi, fr)))
        res[b, :, 0] = re_o
        res[b, :, 1] = im_o
    return res


def kernel(psi_re, psi_im, thetas, u, _trace=False):
    from concourse.bass_utils import run_bass_kernel_spmd

    in_maps = make_in_maps(psi_re, psi_im, thetas, u)
    nc = _get_nc()
    res = run_bass_kernel_spmd(nc, in_maps, list(range(BATCH)), trace=_trace)
    dev_outs = [np.asarray(res.results[b]["out"]) for b in range(BATCH)]
    outs = postprocess(dev_outs, u)
    if _trace:
        return outs, res
    return outs
